# revision 51
# baseline (speedup 1.0000x reference)
"""Multi-head causal attention (B=2, S=2048, D=1024, H=16) on 8 TRN2 NeuronCores.

Sharding: tensor-parallel over heads x data-parallel over batch.
Core c handles batch b = c // 4 and head group g = c % 4 (heads 4g..4g+3),
i.e. a [2048, 256] slice of the output.

Per-core kernel, structured as ONE flat software pipeline (an "atom"
schedule) that starts as x streams in rather than phase-by-phase:

  - DMA order: aux pack -> Wq/Wk(m=0) -> x n0 -> Wv -> x n1 ->
    Wq/Wk(m=1) -> x n2 -> x n3.  The Q/K projections gate the first
    scores (critical path); warmup matmuls (PE HAM clock ramp) cover
    the wait, and attention starts ~18us in.
  - Projections (fp32r, k-contraction 128) are emitted as small quanta
    (one V' s-tile / one Q^T-or-K^T 512-col block) spliced between
    attention windows ~1 per 3, so ScalarE always has an exp queued
    while PE runs projections.
  - Scores/PV run in bf16 (1 cycle/row at ANY moving width, vs fp32r's
    4x penalty under 256).  Scores are computed transposed
    (S^T = K @ Q^T); head pairs share a 128-row Q^T/K^T tile via
    tile_position (0,0)/(64,0) writing the two halves of one [128,1024]
    PSUM tile, sharing a single strided exp on ScalarE.  Causal windows
    are exact; the exp runs on RAW diagonal scores (bounded, finite)
    and the non-causal strip of pt is then zeroed by a 0/1-triangle
    multiply on the otherwise-idle Pool engine (SBUF-only), keeping the
    score->exp chain PE->Act with no VectorE hop.
  - PV matmuls lag the scores by 3 windows GLOBALLY (across block
    boundaries) so PE never waits on exp; each j-block's PVs flush at
    its last window so the denominator ln enters the ScalarE queue
    ahead of the next block's exps.
  - V' tiles carry a ones column per head (DVE memset) so the PV matmul
    also yields softmax denominators; V's bias rides in as a K=1
    ones x bias-row matmul.
  - Normalization: ln(s) on ScalarE -> K=1 ones matmul broadcasts ln(s)
    across 64 partitions -> exp(-x) on ScalarE writes 1/s straight to
    SBUF (no PSUM->SBUF copy) -> one VectorE multiply -> DMA out
    d-major [256, 2048]; the host transposes.  (The custom-DVE
    reciprocal_approx_fast does not encode under this walrus build, and
    GPSIMD cannot touch PSUM, so ScalarE ln/exp it is.)
  - PSUM budget (8 banks): "mm" ring 2 x [128,1024] for score windows
    (4 banks) + "att" ring 4 x [128,512] for PV accumulators, QK
    accumulators, V' tiles and the 1/s broadcasts (4 banks).  aps tiles
    are created at the first PV of a block so ring-slot reuse waits are
    always satisfiable (no rotation deadlock).
"""

import os
import sys

import numpy as np

for _p in ("/opt/trn_rl_repo", "/root/.axon_site/_ro/trn_rl_repo"):
    if os.path.isdir(_p) and _p not in sys.path:
        sys.path.insert(0, _p)

B, S, D, H = 2, 2048, 1024, 16
N_CORES = 8
HEADS_PER_CORE = 4
DH = D // H  # 64
DCORE = HEADS_PER_CORE * DH  # 256
KT = D // 128  # 8 contraction tiles for the projections
ST = S // 128  # 16 sequence tiles
QB = 512  # q block width
NQ = S // QB  # 4 n-blocks
NEG = -1.0e30
AUXC = 900
N_WARM = 86

_CACHE = {}


def _split_multi_waits(nc, max_waits=1):
    """This walrus build rejects instructions carrying more than one
    semaphore wait; hoist extras onto preceding NoOps on the same engine."""
    import bass_rust as _br

    n = 0
    for fn in nc.m.functions:
        for bb in fn.blocks:
            insts = list(bb.instructions)
            new = []
            changed = False
            for inst in insts:
                si = getattr(inst, "sync_info", None)
                ow = list(si.on_wait) if si is not None else []
                if len(ow) > max_waits:
                    changed = True
                    for w in ow[:-max_waits]:
                        n += 1
                        new.append(
                            _br.InstNoOp(
                                name=f"I-ws{n}",
                                engine=inst.engine,
                                ins=[],
                                outs=[],
                                sync_info=_br.SyncInfo(on_wait=[w], on_update=[]),
                            )
                        )
                    si.on_wait = ow[-max_waits:]
                    inst.sync_info = si
                new.append(inst)
            if changed:
                bb.instructions = new


def build_module(repeat=1, hw_loop=False):
    import contextlib

    import concourse.bass as bass
    import concourse.mybir as mybir
    from concourse.tile import TileContext

    F32 = mybir.dt.float32
    F32R = mybir.dt.float32r
    BF16 = mybir.dt.bfloat16
    AF = mybir.ActivationFunctionType

    nc = bass.Bass("TRN2", target_bir_lowering=False, debug=False, num_devices=N_CORES)

    xT_in = nc.declare_dram_parameter("xT", [D, S], F32, isOutput=False)
    wv_in = nc.declare_dram_parameter("wv", [128, KT * DCORE], F32, isOutput=False)
    wqk0_in = nc.declare_dram_parameter("wqk0", [128, 2048], F32, isOutput=False)
    wqk1_in = nc.declare_dram_parameter("wqk1", [128, 2048], F32, isOutput=False)
    aux_in = nc.declare_dram_parameter("aux", [128, AUXC], F32, isOutput=False)
    outT = nc.declare_dram_parameter("outT", [DCORE, S], F32, isOutput=True)

    with TileContext(nc) as tc:
        with (
            tc.tile_pool(name="persist", bufs=1) as pp,
            tc.tile_pool(name="work", bufs=5) as wp,
            tc.tile_pool(name="outp", bufs=3) as op,
            tc.tile_pool(name="mm_ps", bufs=2, space="PSUM") as mm_ps,
            tc.tile_pool(name="att_ps", bufs=4, space="PSUM") as att_ps,
        ):
            # ---- persistent tiles; DMA order is the pipeline schedule ----
            aux = pp.tile([128, AUXC], F32, tag="aux")
            # the whole aux transfer is typed F32R: the BIR verifier
            # tracks fp32r-matmul producers at tensor granularity, and the
            # onesr/bvrow rows feed fp32r matmuls.  F32 consumers (trid,
            # biases) read bit-identical F32 views.
            nc.sync.dma_start(aux[:].bitcast(F32R), aux_in[:].bitcast(F32R))
            wv = pp.tile([128, KT * DCORE], F32R, tag="wv")
            xt = pp.tile([128, KT * S], F32R, tag="xt", name="xt")
            xt_v = xt[:].rearrange("p (k c) -> p k c", c=S)
            xin_v = xT_in[:].rearrange("(k p) c -> p k c", p=128).bitcast(F32R)
            wqk = [
                pp.tile([128, 2048], F32R, tag=f"wqk{m}", name=f"wqk{m}")
                for m in range(2)
            ]

            def load_x_cols(c0, c1):
                nc.sync.dma_start(
                    xt_v[:, :, c0:c1],
                    xin_v[:, :, c0:c1],
                )

            # wqk0 before wv: the Q/K projections gate the first scores (the
            # critical path); V' tiles are only needed once PV starts, ~5us
            # later.  Warmup matmuls cover the x wait.
            nc.sync.dma_start(wqk[0][:], wqk0_in[:].bitcast(F32R))
            load_x_cols(0, QB)
            nc.sync.dma_start(wv[:], wv_in[:].bitcast(F32R))
            load_x_cols(QB, 2 * QB)
            nc.sync.dma_start(wqk[1][:], wqk1_in[:].bitcast(F32R))
            load_x_cols(2 * QB, 3 * QB)
            load_x_cols(3 * QB, 4 * QB)

            trid = aux[:, 0:256]
            onesr = aux[0:1, 256:384].bitcast(F32R)
            bvrow = aux[0:1, 384:640].bitcast(F32R)
            bqc = aux[:, 640:642]
            bkc = aux[:, 642:644]

            # ---- warmup during the DMA window: ramps the PE HAM clock
            # gate to 2.4 GHz; one exp pulls the table load off the
            # critical path ------------------------------------------------
            warm_ps = mm_ps.tile([128, 2 * QB], F32, tag="mm", name="warm_ps")
            for _w in range(N_WARM):
                nc.tensor.matmul(
                    warm_ps[:, 0:DCORE], onesr[:], bvrow[:], start=True, stop=True
                )
            tri01 = pp.tile([128, 256], BF16, tag="tri01")
            nc.vector.tensor_copy(tri01[:], aux[:, 644:900])
            warm_o = wp.tile([1, 128], F32, tag="lns", name="warm_o")
            nc.scalar.activation(warm_o[:], onesr[:].bitcast(F32), AF.Exp)
            nc.scalar.activation(warm_o[:], warm_o[:], AF.Ln)

            if hw_loop and repeat > 1:
                rep_iter = [0]
                rep_ctx = tc.For_i(0, repeat, 1)
            else:
                rep_iter = range(repeat)
                rep_ctx = contextlib.nullcontext()
            with rep_ctx:
              for _rep in rep_iter:
                qT = [
                    pp.tile([128, S], BF16, tag=f"qT{m}", name=f"qT{m}")
                    for m in range(2)
                ]
                kTt = [
                    pp.tile([128, S], BF16, tag=f"kT{m}", name=f"kT{m}")
                    for m in range(2)
                ]
                vp = [
                    pp.tile([128, 4 * 65], BF16, tag=f"vp{s}", name=f"vp{s}")
                    for s in range(ST)
                ]

                def v_tile(s):
                    # single V' s-tile (needs x n-block s//4)
                    dst = vp[s]
                    nc.vector.memset(
                        dst[:].rearrange("p (h c) -> p h c", c=65)[:, :, 64:65], 1.0
                    )
                    ps = att_ps.tile([128, DCORE], F32, tag="att", name=f"vps{s}")
                    for k in range(KT):
                        nc.tensor.matmul(
                            ps[:],
                            xt[:, S * k + 128 * s : S * k + 128 * (s + 1)],
                            wv[:, DCORE * k : DCORE * (k + 1)],
                            start=(k == 0),
                            stop=False,
                        )
                    nc.tensor.matmul(ps[:], onesr[:], bvrow[:], start=False, stop=True)
                    nc.vector.tensor_copy(
                        dst[:].rearrange("p (h c) -> p h c", c=65)[:, :, 0:64],
                        ps[:].rearrange("p (h c) -> p h c", c=64),
                    )

                def qk_half(n, m, which, ring=None):
                    # one Q^T or K^T 512-col projection block (needs x block n
                    # and wqk[m]); wqk[m] holds q k-tiles then k k-tiles.
                    nwin = slice(QB * n, QB * (n + 1))
                    dst, woff, bias = (
                        (qT, 0, bqc) if which == 0 else (kTt, 1024, bkc)
                    )
                    pool, tag = (mm_ps, "mm") if ring == "mm" else (att_ps, "att")
                    acc = pool.tile([128, QB], F32, tag=tag, name=f"qk{n}{m}{which}")
                    for k in range(KT):
                        nc.tensor.matmul(
                            acc[:],
                            wqk[m][:, woff + 128 * k : woff + 128 * (k + 1)],
                            xt[:, S * k + QB * n : S * k + QB * (n + 1)],
                            start=(k == 0),
                            stop=(k == KT - 1),
                        )
                    nc.vector.tensor_scalar_add(
                        dst[m][:, nwin], acc[:], bias[:, m : m + 1]
                    )

                def scores(hp, j, t):
                    # one score window: both heads of pair hp, k-tile t,
                    # q block j; returns (pt, qoff) for the PV stage
                    i = t - 4 * j
                    qoff = 128 * max(i, 0)
                    qwin = slice(QB * j + qoff, QB * (j + 1))
                    ktile = slice(128 * t, 128 * (t + 1))
                    sps = mm_ps.tile([128, 2 * QB], F32, tag="mm", name="sps")
                    spsv = sps[:].rearrange("p (two c) -> p two c", two=2)
                    nc.tensor.matmul(
                        sps[:, qoff:QB],
                        kTt[hp][0:64, ktile],
                        qT[hp][0:64, qwin],
                        start=True,
                        stop=True,
                        tile_position=(0, 0),
                    )
                    nc.tensor.matmul(
                        sps[:, QB + qoff : 2 * QB],
                        kTt[hp][64:128, ktile],
                        qT[hp][64:128, qwin],
                        start=True,
                        stop=True,
                        tile_position=(64, 0),
                    )
                    pt = wp.tile([128, 2 * QB], BF16, tag="pt")
                    ptv = pt[:].rearrange("p (two c) -> p two c", two=2)
                    nc.scalar.activation(
                        ptv[:, :, qoff:QB],
                        spsv[:, :, qoff:QB],
                        AF.Exp,
                        scale=float(1.0 / np.sqrt(DH)),
                    )
                    if i >= 0:
                        # zero the strictly-upper (non-causal) part of the
                        # diagonal strip: raw scores are bounded so exp is
                        # finite; the 0/1-triangle multiply runs on the idle
                        # Pool engine (SBUF-only), keeping the score->exp
                        # chain PE->Act with no DVE hop
                        nc.gpsimd.tensor_mul(
                            ptv[:, :, qoff : qoff + 128],
                            ptv[:, :, qoff : qoff + 128],
                            tri01[:].rearrange("p (two c) -> p two c", two=2),
                        )
                    return pt, qoff

                def pv(hp, j, t, pt, qoff, aps):
                    for hl, off in ((2 * hp, 0), (2 * hp + 1, QB)):
                        nc.tensor.matmul(
                            aps[hl % 2][0:65, qoff:QB],
                            vp[t][:, 65 * hl : 65 * hl + 65],
                            pt[:, off + qoff : off + QB],
                            start=(t == 0),
                            stop=(t == 4 * j + 3),
                        )

                def recips(aps):
                    # 1/denominator rows; issued right after the block's last
                    # PV so the DVE work overlaps the next block's scores
                    # ln(s) rows on ScalarE (the custom-DVE reciprocal
                    # does not encode under this walrus build; plain DVE
                    # reciprocal runs ~7 cycles/element).  The exp(-ln s)
                    # happens AFTER the partition broadcast, writing the
                    # reciprocal straight to SBUF -- the PE broadcast only
                    # waits for the ln, and no PSUM->SBUF copy is needed.
                    rrows = []
                    for a in range(2):
                        lns = wp.tile([1, QB], F32R, tag="lns", name=f"lns{a}")
                        nc.scalar.activation(lns[:], aps[a][64:65, 0:QB], AF.Ln)
                        rrows.append(lns)
                    return rrows

                def norm_rest(hp, j, aps, rrows):
                    rbs = []
                    for a in range(2):
                        rbp = att_ps.tile([64, QB], F32, tag="att", name=f"rbp{a}")
                        nc.tensor.matmul(
                            rbp[:],
                            onesr[:, 0:64],
                            rrows[a][:],
                            start=True,
                            stop=True,
                        )
                        rb = wp.tile([64, QB], F32, tag="rb", name="rb")
                        nc.scalar.activation(rb[:], rbp[:], AF.Exp, scale=-1.0)
                        rbs.append(rb)
                    for a in range(2):
                        h = 2 * hp + a
                        att = op.tile([64, QB], F32, tag="att_out", name="att")
                        nc.vector.tensor_mul(att[:], aps[a][0:64, :], rbs[a][:])
                        nc.sync.dma_start(
                            outT[64 * h : 64 * (h + 1), QB * j : QB * (j + 1)],
                            att[:],
                        )

                # ---- the pipeline: a flat atom schedule.  Attention blocks
                # are broken into windows ("w"); projection quanta ("v" tile /
                # "qk" half-block) are spliced in ~1 per 3 windows so ScalarE
                # always has an exp queued while PE runs projections; each
                # block's normalize ("n") is deferred 2 windows into the next
                # block so its PE/DVE chain hides in the pipeline ------------
                def att_atoms(hp, j, fill):
                    # all fills must land before the block's last window: the
                    # PV flush at w(last) consumes every vp[t] of the block
                    out = []
                    fi = 0
                    for t in range(4 * j + 4):
                        out.append(("w", hp, j, t))
                        if t == 4 * j + 2:
                            out.extend(fill[fi:])
                            fi = len(fill)
                        elif fi < len(fill) and t % 3 == 1:
                            out.append(fill[fi])
                            fi += 1
                    return out

                # Fill-placement constraint: ("v", s) must appear before any
                # window atom that reads vp[s] (a block's own fills are safe
                # through its last window since PV lags scores by 2).
                atoms = []
                atoms += [("qk", 0, 0, 0, "mm"), ("qk", 0, 0, 1, "mm")]
                atoms += [("v", 0), ("v", 1)]
                atoms += att_atoms(0, 0, [("v", 2), ("v", 3)])
                atoms += [("qk", 1, 0, 0), ("qk", 1, 0, 1)]
                atoms += att_atoms(0, 1, [("v", 4), ("v", 5), ("v", 6), ("v", 7)])
                atoms += [("qk", 0, 1, 0), ("qk", 0, 1, 1)]
                atoms += att_atoms(1, 0, [])
                atoms += [("qk", 2, 0, 0), ("qk", 2, 0, 1)]
                atoms += att_atoms(
                    0,
                    2,
                    [
                        ("v", 8),
                        ("v", 9),
                        ("qk", 1, 1, 0),
                        ("qk", 1, 1, 1),
                        ("v", 10),
                        ("v", 11),
                    ],
                )
                atoms += att_atoms(1, 1, [])
                atoms += [("qk", 3, 0, 0), ("qk", 3, 0, 1)]
                atoms += att_atoms(
                    0,
                    3,
                    [
                        ("v", 12),
                        ("v", 13),
                        ("v", 14),
                        ("v", 15),
                        ("qk", 2, 1, 0),
                    ],
                )
                atoms += att_atoms(
                    1, 2, [("qk", 2, 1, 1), ("qk", 3, 1, 0), ("qk", 3, 1, 1)]
                )
                atoms += att_atoms(1, 3, [])

                # splice each block's "n" atom 2 windows into the next block
                final = []
                pending_norm = []
                wcount = 0
                for a in atoms:
                    final.append(a)
                    if a[0] == "w":
                        hp, j, t = a[1], a[2], a[3]
                        if pending_norm:
                            wcount += 1
                            if wcount == 2:
                                final.append(pending_norm.pop(0))
                        if t == 4 * j + 3:
                            pending_norm.append(("n", hp, j))
                            wcount = 0
                final.extend(pending_norm)

                # Window processing: PV lags scores by 2 windows GLOBALLY
                # (across block boundaries) so PE never waits on exp and the
                # block-end PV drain interleaves with the next block's scores.
                # aps tiles are created at the first PV of a block so the
                # "att" ring allocation order keeps every slot-reuse wait
                # satisfiable (no rotation deadlock).
                state = {}
                pend = []

                def emit_pv(p):
                    hp, j, t, pt, qoff = p
                    st = state[(hp, j)]
                    if st["aps"] is None:
                        st["aps"] = [
                            att_ps.tile(
                                [128, QB], F32, tag="att", name=f"aps{hp}_{j}_{x}"
                            )
                            for x in range(2)
                        ]
                    pv(hp, j, t, pt, qoff, st["aps"])
                    if t == 4 * j + 3:
                        st["rrows"] = recips(st["aps"])

                for a in final:
                    if a[0] == "v":
                        v_tile(a[1])
                    elif a[0] == "qk":
                        qk_half(a[1], a[2], a[3], a[4] if len(a) > 4 else None)
                    elif a[0] == "w":
                        hp, j, t = a[1], a[2], a[3]
                        state.setdefault((hp, j), {"aps": None})
                        pt, qoff = scores(hp, j, t)
                        pend.append((hp, j, t, pt, qoff))
                        if t == 4 * j + 3:
                            while pend:
                                emit_pv(pend.pop(0))
                        elif len(pend) > 3:
                            emit_pv(pend.pop(0))
                    else:
                        hp, j = a[1], a[2]
                        while pend and (pend[0][0], pend[0][1]) == (hp, j):
                            emit_pv(pend.pop(0))
                        st = state[(hp, j)]
                        norm_rest(hp, j, st["aps"], st["rrows"])

    _split_multi_waits(nc)
    return nc


def _get_runner():
    if "nc" not in _CACHE:
        _CACHE["nc"] = build_module()
    return _CACHE["nc"]


def _pack_w(Wcol):
    # [1024, C] -> [128, 8*C] bf16 with k-tiles along the free dim
    C = Wcol.shape[1]
    return np.ascontiguousarray(
        Wcol.reshape(KT, 128, C).transpose(1, 0, 2).reshape(128, KT * C)
    )


def _make_in_maps(x, Wq, bq, Wk, bk, Wv, bv):
    x = np.asarray(x, dtype=np.float32)
    Wq = np.asarray(Wq, dtype=np.float32)
    Wk = np.asarray(Wk, dtype=np.float32)
    Wv = np.asarray(Wv, dtype=np.float32)
    bq = np.asarray(bq, dtype=np.float32)
    bk = np.asarray(bk, dtype=np.float32)
    bv = np.asarray(bv, dtype=np.float32)

    kp = np.arange(128)[:, None]
    qf = np.arange(128)[None, :]
    tri = np.where(kp <= qf, 0.0, NEG).astype(np.float32)

    xTs = [np.ascontiguousarray(x[b].T) for b in range(B)]
    in_maps = []
    for c in range(N_CORES):
        b = c // 4
        g = c % 4
        sl = slice(DCORE * g, DCORE * (g + 1))
        wq_g = Wq[:, sl]
        wk_g = Wk[:, sl]
        aux = np.zeros((128, AUXC), np.float32)
        aux[:, 0:128] = tri
        aux[:, 128:256] = tri
        tri01 = np.where(kp <= qf, 1.0, 0.0).astype(np.float32)
        aux[:, 644:772] = tri01
        aux[:, 772:900] = tri01
        aux[0, 256:384] = 1.0
        aux[0, 384:640] = bv[sl]
        aux[:, 640] = bq[sl][0:128]
        aux[:, 641] = bq[sl][128:256]
        aux[:, 642] = bk[sl][0:128]
        aux[:, 643] = bk[sl][128:256]
        in_maps.append(
            {
                "xT": xTs[b],
                "wv": _pack_w(Wv[:, sl]),
                "wqk0": np.concatenate(
                    [_pack_w(wq_g[:, 0:128]), _pack_w(wk_g[:, 0:128])], axis=1
                ),
                "wqk1": np.concatenate(
                    [_pack_w(wq_g[:, 128:256]), _pack_w(wk_g[:, 128:256])], axis=1
                ),
                "aux": aux,
            }
        )
    return in_maps


def kernel(x, Wq, bq, Wk, bk, Wv, bv):
    from concourse.bass_utils import run_bass_kernel_spmd

    nc = _get_runner()
    in_maps = _make_in_maps(x, Wq, bq, Wk, bk, Wv, bv)
    res = run_bass_kernel_spmd(nc, in_maps, list(range(N_CORES)))
    out = np.empty((B, S, D), dtype=np.float32)
    for c in range(N_CORES):
        b = c // 4
        g = c % 4
        out[b, :, DCORE * g : DCORE * (g + 1)] = res.results[c]["outT"].T
    return out


# revision 53
# speedup vs baseline: 2.5008x; 2.5008x over previous
"""Multi-head causal attention (B=2, S=2048, D=1024, H=16) on 8 TRN2 NeuronCores.

Sharding: tensor-parallel over heads x data-parallel over batch.
Core c handles batch b = c // 4 and head group g = c % 4 (heads 4g..4g+3),
i.e. a [2048, 256] slice of the output.

Per-core kernel, structured as ONE flat software pipeline (an "atom"
schedule) that starts as x streams in rather than phase-by-phase:

  - DMA order: aux pack -> Wq/Wk(m=0) -> x n0 -> Wv -> x n1 ->
    Wq/Wk(m=1) -> x n2 -> x n3.  The Q/K projections gate the first
    scores (critical path); warmup matmuls (PE HAM clock ramp) cover
    the wait, and attention starts ~18us in.
  - Projections (fp32r, k-contraction 128) are emitted as small quanta
    (one V' s-tile / one Q^T-or-K^T 512-col block) spliced between
    attention windows ~1 per 3, so ScalarE always has an exp queued
    while PE runs projections.
  - Scores/PV run in bf16 (1 cycle/row at ANY moving width, vs fp32r's
    4x penalty under 256).  Scores are computed transposed
    (S^T = K @ Q^T); head pairs share a 128-row Q^T/K^T tile via
    tile_position (0,0)/(64,0) writing the two halves of one [128,1024]
    PSUM tile, sharing a single strided exp on ScalarE.  Causal windows
    are exact; the exp runs on RAW diagonal scores (bounded, finite)
    and the non-causal strip of pt is then zeroed by a 0/1-triangle
    multiply on the otherwise-idle Pool engine (SBUF-only), keeping the
    score->exp chain PE->Act with no VectorE hop.
  - PV matmuls lag the scores by 3 windows GLOBALLY (across block
    boundaries) so PE never waits on exp; each j-block's PVs flush at
    its last window so the denominator ln enters the ScalarE queue
    ahead of the next block's exps.
  - V' tiles carry a ones column per head (DVE memset) so the PV matmul
    also yields softmax denominators; V's bias rides in as a K=1
    ones x bias-row matmul.
  - Normalization: ln(s) on ScalarE -> K=1 ones matmul broadcasts ln(s)
    across 64 partitions -> exp(-x) on ScalarE writes 1/s straight to
    SBUF (no PSUM->SBUF copy) -> one VectorE multiply -> DMA out
    d-major [256, 2048]; the host transposes.  (The custom-DVE
    reciprocal_approx_fast does not encode under this walrus build, and
    GPSIMD cannot touch PSUM, so ScalarE ln/exp it is.)
  - PSUM budget (8 banks): "mm" ring 2 x [128,1024] for score windows
    (4 banks) + "att" ring 4 x [128,512] for PV accumulators, QK
    accumulators, V' tiles and the 1/s broadcasts (4 banks).  aps tiles
    are created at the first PV of a block so ring-slot reuse waits are
    always satisfiable (no rotation deadlock).
"""

import os
import sys

import numpy as np

for _p in ("/opt/trn_rl_repo", "/root/.axon_site/_ro/trn_rl_repo"):
    if os.path.isdir(_p) and _p not in sys.path:
        sys.path.insert(0, _p)

B, S, D, H = 2, 2048, 1024, 16
N_CORES = 8
HEADS_PER_CORE = 4
DH = D // H  # 64
DCORE = HEADS_PER_CORE * DH  # 256
KT = D // 128  # 8 contraction tiles for the projections
ST = S // 128  # 16 sequence tiles
QB = 512  # q block width
NQ = S // QB  # 4 n-blocks
NEG = -1.0e30
AUXC = 900
N_WARM = 86

_CACHE = {}


def _split_multi_waits(nc, max_waits=1):
    """This walrus build rejects instructions carrying more than one
    semaphore wait; hoist extras onto preceding NoOps on the same engine."""
    import bass_rust as _br

    n = 0
    for fn in nc.m.functions:
        for bb in fn.blocks:
            insts = list(bb.instructions)
            new = []
            changed = False
            for inst in insts:
                si = getattr(inst, "sync_info", None)
                ow = list(si.on_wait) if si is not None else []
                if len(ow) > max_waits:
                    changed = True
                    for w in ow[:-max_waits]:
                        n += 1
                        new.append(
                            _br.InstNoOp(
                                name=f"I-ws{n}",
                                engine=inst.engine,
                                ins=[],
                                outs=[],
                                sync_info=_br.SyncInfo(on_wait=[w], on_update=[]),
                            )
                        )
                    si.on_wait = ow[-max_waits:]
                    inst.sync_info = si
                new.append(inst)
            if changed:
                bb.instructions = new


def build_module(repeat=1, hw_loop=False):
    import contextlib

    import concourse.bass as bass
    import concourse.mybir as mybir
    from concourse.tile import TileContext

    F32 = mybir.dt.float32
    F32R = mybir.dt.float32r
    BF16 = mybir.dt.bfloat16
    AF = mybir.ActivationFunctionType

    nc = bass.Bass("TRN2", target_bir_lowering=False, debug=False, num_devices=N_CORES)

    xT_in = nc.declare_dram_parameter("xT", [D, S], F32, isOutput=False)
    wv_in = nc.declare_dram_parameter("wv", [128, KT * DCORE], F32, isOutput=False)
    wqk0_in = nc.declare_dram_parameter("wqk0", [128, 2048], F32, isOutput=False)
    wqk1_in = nc.declare_dram_parameter("wqk1", [128, 2048], F32, isOutput=False)
    aux_in = nc.declare_dram_parameter("aux", [128, AUXC], F32, isOutput=False)
    outT = nc.declare_dram_parameter("outT", [DCORE, S], F32, isOutput=True)

    with TileContext(nc) as tc:
        with (
            tc.tile_pool(name="persist", bufs=1) as pp,
            tc.tile_pool(name="work", bufs=5) as wp,
            tc.tile_pool(name="outp", bufs=3) as op,
            tc.tile_pool(name="mm_ps", bufs=2, space="PSUM") as mm_ps,
            tc.tile_pool(name="att_ps", bufs=4, space="PSUM") as att_ps,
        ):
            # ---- persistent tiles; DMA order is the pipeline schedule ----
            aux = pp.tile([128, AUXC], F32, tag="aux")
            # the whole aux transfer is typed F32R: the BIR verifier
            # tracks fp32r-matmul producers at tensor granularity, and the
            # onesr/bvrow rows feed fp32r matmuls.  F32 consumers (trid,
            # biases) read bit-identical F32 views.
            nc.sync.dma_start(aux[:].bitcast(F32R), aux_in[:].bitcast(F32R))
            wv = pp.tile([128, KT * DCORE], F32R, tag="wv")
            xt = pp.tile([128, KT * S], F32R, tag="xt", name="xt")
            xt_v = xt[:].rearrange("p (k c) -> p k c", c=S)
            xin_v = xT_in[:].rearrange("(k p) c -> p k c", p=128).bitcast(F32R)
            wqk = [
                pp.tile([128, 2048], F32R, tag=f"wqk{m}", name=f"wqk{m}")
                for m in range(2)
            ]

            def load_x_cols(c0, c1):
                nc.sync.dma_start(
                    xt_v[:, :, c0:c1],
                    xin_v[:, :, c0:c1],
                )

            # wqk0 before wv: the Q/K projections gate the first scores (the
            # critical path); V' tiles are only needed once PV starts, ~5us
            # later.  Warmup matmuls cover the x wait.
            nc.sync.dma_start(wqk[0][:], wqk0_in[:].bitcast(F32R))
            load_x_cols(0, QB)
            nc.sync.dma_start(wv[:], wv_in[:].bitcast(F32R))
            load_x_cols(QB, 2 * QB)
            nc.sync.dma_start(wqk[1][:], wqk1_in[:].bitcast(F32R))
            load_x_cols(2 * QB, 3 * QB)
            load_x_cols(3 * QB, 4 * QB)

            trid = aux[:, 0:256]
            onesr = aux[0:1, 256:384].bitcast(F32R)
            bvrow = aux[0:1, 384:640].bitcast(F32R)
            bqc = aux[:, 640:642]
            bkc = aux[:, 642:644]

            # ---- warmup during the DMA window: ramps the PE HAM clock
            # gate to 2.4 GHz; one exp pulls the table load off the
            # critical path ------------------------------------------------
            warm_ps = mm_ps.tile([128, 2 * QB], F32, tag="mm", name="warm_ps")
            for _w in range(N_WARM):
                nc.tensor.matmul(
                    warm_ps[:, 0:DCORE], onesr[:], bvrow[:], start=True, stop=True
                )
            tri01 = pp.tile([128, 256], BF16, tag="tri01")
            nc.vector.tensor_copy(tri01[:], aux[:, 644:900])
            warm_o = wp.tile([1, 128], F32, tag="lns", name="warm_o")
            nc.scalar.activation(warm_o[:], onesr[:].bitcast(F32), AF.Exp)
            nc.scalar.activation(warm_o[:], warm_o[:], AF.Ln)

            if hw_loop and repeat > 1:
                rep_iter = [0]
                rep_ctx = tc.For_i(0, repeat, 1)
            else:
                rep_iter = range(repeat)
                rep_ctx = contextlib.nullcontext()
            with rep_ctx:
              for _rep in rep_iter:
                qT = [
                    pp.tile([128, S], BF16, tag=f"qT{m}", name=f"qT{m}")
                    for m in range(2)
                ]
                kTt = [
                    pp.tile([128, S], BF16, tag=f"kT{m}", name=f"kT{m}")
                    for m in range(2)
                ]
                vp = [
                    pp.tile([128, 4 * 65], BF16, tag=f"vp{s}", name=f"vp{s}")
                    for s in range(ST)
                ]

                def v_tile(s):
                    # single V' s-tile (needs x n-block s//4)
                    dst = vp[s]
                    nc.vector.memset(
                        dst[:].rearrange("p (h c) -> p h c", c=65)[:, :, 64:65], 1.0
                    )
                    ps = att_ps.tile([128, DCORE], F32, tag="att", name=f"vps{s}")
                    for k in range(KT):
                        nc.tensor.matmul(
                            ps[:],
                            xt[:, S * k + 128 * s : S * k + 128 * (s + 1)],
                            wv[:, DCORE * k : DCORE * (k + 1)],
                            start=(k == 0),
                            stop=False,
                        )
                    nc.tensor.matmul(ps[:], onesr[:], bvrow[:], start=False, stop=True)
                    nc.vector.tensor_copy(
                        dst[:].rearrange("p (h c) -> p h c", c=65)[:, :, 0:64],
                        ps[:].rearrange("p (h c) -> p h c", c=64),
                    )

                def qk_half(n, m, which, ring=None):
                    # one Q^T or K^T 512-col projection block (needs x block n
                    # and wqk[m]); wqk[m] holds q k-tiles then k k-tiles.
                    nwin = slice(QB * n, QB * (n + 1))
                    dst, woff, bias = (
                        (qT, 0, bqc) if which == 0 else (kTt, 1024, bkc)
                    )
                    pool, tag = (mm_ps, "mm") if ring == "mm" else (att_ps, "att")
                    acc = pool.tile([128, QB], F32, tag=tag, name=f"qk{n}{m}{which}")
                    for k in range(KT):
                        nc.tensor.matmul(
                            acc[:],
                            wqk[m][:, woff + 128 * k : woff + 128 * (k + 1)],
                            xt[:, S * k + QB * n : S * k + QB * (n + 1)],
                            start=(k == 0),
                            stop=(k == KT - 1),
                        )
                    nc.vector.tensor_scalar_add(
                        dst[m][:, nwin], acc[:], bias[:, m : m + 1]
                    )

                def scores(hp, j, t):
                    # one score window: both heads of pair hp, k-tile t,
                    # q block j; returns (pt, qoff) for the PV stage
                    i = t - 4 * j
                    qoff = 128 * max(i, 0)
                    qwin = slice(QB * j + qoff, QB * (j + 1))
                    ktile = slice(128 * t, 128 * (t + 1))
                    sps = mm_ps.tile([128, 2 * QB], F32, tag="mm", name="sps")
                    spsv = sps[:].rearrange("p (two c) -> p two c", two=2)
                    nc.tensor.matmul(
                        sps[:, qoff:QB],
                        kTt[hp][0:64, ktile],
                        qT[hp][0:64, qwin],
                        start=True,
                        stop=True,
                        tile_position=(0, 0),
                    )
                    nc.tensor.matmul(
                        sps[:, QB + qoff : 2 * QB],
                        kTt[hp][64:128, ktile],
                        qT[hp][64:128, qwin],
                        start=True,
                        stop=True,
                        tile_position=(64, 0),
                    )
                    pt = wp.tile([128, 2 * QB], BF16, tag="pt")
                    ptv = pt[:].rearrange("p (two c) -> p two c", two=2)
                    nc.scalar.activation(
                        ptv[:, :, qoff:QB],
                        spsv[:, :, qoff:QB],
                        AF.Exp,
                        scale=float(1.0 / np.sqrt(DH)),
                    )
                    if i >= 0:
                        # zero the strictly-upper (non-causal) part of the
                        # diagonal strip: raw scores are bounded so exp is
                        # finite; the 0/1-triangle multiply runs on the idle
                        # Pool engine (SBUF-only), keeping the score->exp
                        # chain PE->Act with no VectorE hop
                        nc.gpsimd.tensor_mul(
                            ptv[:, :, qoff : qoff + 128],
                            ptv[:, :, qoff : qoff + 128],
                            tri01[:].rearrange("p (two c) -> p two c", two=2),
                        )
                    return pt, qoff

                def pv(hp, j, t, pt, qoff, aps):
                    for hl, off in ((2 * hp, 0), (2 * hp + 1, QB)):
                        nc.tensor.matmul(
                            aps[hl % 2][0:65, qoff:QB],
                            vp[t][:, 65 * hl : 65 * hl + 65],
                            pt[:, off + qoff : off + QB],
                            start=(t == 0),
                            stop=(t == 4 * j + 3),
                        )

                def recips(aps):
                    # 1/denominator rows; issued right after the block's last
                    # PV so the DVE work overlaps the next block's scores
                    # ln(s) rows on ScalarE (the custom-DVE reciprocal
                    # does not encode under this walrus build; plain DVE
                    # reciprocal runs ~7 cycles/element).  The exp(-ln s)
                    # happens AFTER the partition broadcast, writing the
                    # reciprocal straight to SBUF -- the PE broadcast only
                    # waits for the ln, and no PSUM->SBUF copy is needed.
                    rrows = []
                    for a in range(2):
                        lns = wp.tile([1, QB], F32R, tag="lns", name=f"lns{a}")
                        nc.scalar.activation(lns[:], aps[a][64:65, 0:QB], AF.Ln)
                        rrows.append(lns)
                    return rrows

                def norm_rest(hp, j, aps, rrows):
                    rbs = []
                    for a in range(2):
                        rbp = att_ps.tile([64, QB], F32, tag="att", name=f"rbp{a}")
                        nc.tensor.matmul(
                            rbp[:],
                            onesr[:, 0:64],
                            rrows[a][:],
                            start=True,
                            stop=True,
                        )
                        rb = wp.tile([64, QB], F32, tag="rb", name="rb")
                        nc.scalar.activation(rb[:], rbp[:], AF.Exp, scale=-1.0)
                        rbs.append(rb)
                    for a in range(2):
                        h = 2 * hp + a
                        att = op.tile([64, QB], F32, tag="att_out", name="att")
                        nc.vector.tensor_mul(att[:], aps[a][0:64, :], rbs[a][:])
                        nc.sync.dma_start(
                            outT[64 * h : 64 * (h + 1), QB * j : QB * (j + 1)],
                            att[:],
                        )

                # ---- the pipeline: a flat atom schedule.  Attention blocks
                # are broken into windows ("w"); projection quanta ("v" tile /
                # "qk" half-block) are spliced in ~1 per 3 windows so ScalarE
                # always has an exp queued while PE runs projections; each
                # block's normalize ("n") is deferred 2 windows into the next
                # block so its PE/DVE chain hides in the pipeline ------------
                def att_atoms(hp, j, fill):
                    # all fills must land before the block's last window: the
                    # PV flush at w(last) consumes every vp[t] of the block
                    out = []
                    fi = 0
                    for t in range(4 * j + 4):
                        out.append(("w", hp, j, t))
                        if t == 4 * j + 2:
                            out.extend(fill[fi:])
                            fi = len(fill)
                        elif fi < len(fill) and t % 3 == 1:
                            out.append(fill[fi])
                            fi += 1
                    return out

                # Fill-placement constraint: ("v", s) must appear before any
                # window atom that reads vp[s] (a block's own fills are safe
                # through its last window since PV lags scores by 2).
                atoms = []
                atoms += [("qk", 0, 0, 0, "mm"), ("qk", 0, 0, 1, "mm")]
                atoms += [("v", 0), ("v", 1)]
                atoms += att_atoms(0, 0, [("v", 2), ("v", 3)])
                atoms += [("qk", 1, 0, 0), ("qk", 1, 0, 1)]
                atoms += att_atoms(0, 1, [("v", 4), ("v", 5), ("v", 6), ("v", 7)])
                atoms += [("qk", 0, 1, 0), ("qk", 0, 1, 1)]
                atoms += att_atoms(1, 0, [])
                atoms += [("qk", 2, 0, 0), ("qk", 2, 0, 1)]
                atoms += att_atoms(
                    0,
                    2,
                    [
                        ("v", 8),
                        ("v", 9),
                        ("qk", 1, 1, 0),
                        ("qk", 1, 1, 1),
                        ("v", 10),
                        ("v", 11),
                    ],
                )
                atoms += att_atoms(1, 1, [])
                atoms += [("qk", 3, 0, 0), ("qk", 3, 0, 1)]
                atoms += att_atoms(
                    0,
                    3,
                    [
                        ("v", 12),
                        ("v", 13),
                        ("v", 14),
                        ("v", 15),
                        ("qk", 2, 1, 0),
                    ],
                )
                atoms += att_atoms(
                    1, 2, [("qk", 2, 1, 1), ("qk", 3, 1, 0), ("qk", 3, 1, 1)]
                )
                atoms += att_atoms(1, 3, [])

                # splice each block's "n" atom 2 windows into the next block
                final = []
                pending_norm = []
                wcount = 0
                for a in atoms:
                    final.append(a)
                    if a[0] == "w":
                        hp, j, t = a[1], a[2], a[3]
                        if pending_norm:
                            wcount += 1
                            if wcount == 2:
                                final.append(pending_norm.pop(0))
                        if t == 4 * j + 3:
                            pending_norm.append(("n", hp, j))
                            wcount = 0
                final.extend(pending_norm)

                # Window processing: PV lags scores by 2 windows GLOBALLY
                # (across block boundaries) so PE never waits on exp and the
                # block-end PV drain interleaves with the next block's scores.
                # aps tiles are created at the first PV of a block so the
                # "att" ring allocation order keeps every slot-reuse wait
                # satisfiable (no rotation deadlock).
                state = {}
                pend = []

                def emit_pv(p):
                    hp, j, t, pt, qoff = p
                    st = state[(hp, j)]
                    if st["aps"] is None:
                        st["aps"] = [
                            att_ps.tile(
                                [128, QB], F32, tag="att", name=f"aps{hp}_{j}_{x}"
                            )
                            for x in range(2)
                        ]
                    pv(hp, j, t, pt, qoff, st["aps"])
                    if t == 4 * j + 3:
                        st["rrows"] = recips(st["aps"])

                for a in final:
                    if a[0] == "v":
                        v_tile(a[1])
                    elif a[0] == "qk":
                        qk_half(a[1], a[2], a[3], a[4] if len(a) > 4 else None)
                    elif a[0] == "w":
                        hp, j, t = a[1], a[2], a[3]
                        state.setdefault((hp, j), {"aps": None})
                        pt, qoff = scores(hp, j, t)
                        pend.append((hp, j, t, pt, qoff))
                        if t == 4 * j + 3:
                            while pend:
                                emit_pv(pend.pop(0))
                        elif len(pend) > 3:
                            emit_pv(pend.pop(0))
                    else:
                        hp, j = a[1], a[2]
                        while pend and (pend[0][0], pend[0][1]) == (hp, j):
                            emit_pv(pend.pop(0))
                        st = state[(hp, j)]
                        norm_rest(hp, j, st["aps"], st["rrows"])

    _split_multi_waits(nc)
    return nc


def _get_runner():
    if "nc" not in _CACHE:
        _CACHE["nc"] = build_module()
    return _CACHE["nc"]


def _pack_w(Wcol):
    # [1024, C] -> [128, 8*C] bf16 with k-tiles along the free dim
    C = Wcol.shape[1]
    return np.ascontiguousarray(
        Wcol.reshape(KT, 128, C).transpose(1, 0, 2).reshape(128, KT * C)
    )


def _make_in_maps(x, Wq, bq, Wk, bk, Wv, bv):
    x = np.asarray(x, dtype=np.float32)
    Wq = np.asarray(Wq, dtype=np.float32)
    Wk = np.asarray(Wk, dtype=np.float32)
    Wv = np.asarray(Wv, dtype=np.float32)
    bq = np.asarray(bq, dtype=np.float32)
    bk = np.asarray(bk, dtype=np.float32)
    bv = np.asarray(bv, dtype=np.float32)

    kp = np.arange(128)[:, None]
    qf = np.arange(128)[None, :]
    tri = np.where(kp <= qf, 0.0, NEG).astype(np.float32)

    xTs = [np.ascontiguousarray(x[b].T) for b in range(B)]
    in_maps = []
    for c in range(N_CORES):
        b = c // 4
        g = c % 4
        sl = slice(DCORE * g, DCORE * (g + 1))
        wq_g = Wq[:, sl]
        wk_g = Wk[:, sl]
        aux = np.zeros((128, AUXC), np.float32)
        aux[:, 0:128] = tri
        aux[:, 128:256] = tri
        tri01 = np.where(kp <= qf, 1.0, 0.0).astype(np.float32)
        aux[:, 644:772] = tri01
        aux[:, 772:900] = tri01
        aux[0, 256:384] = 1.0
        aux[0, 384:640] = bv[sl]
        aux[:, 640] = bq[sl][0:128]
        aux[:, 641] = bq[sl][128:256]
        aux[:, 642] = bk[sl][0:128]
        aux[:, 643] = bk[sl][128:256]
        in_maps.append(
            {
                "xT": xTs[b],
                "wv": _pack_w(Wv[:, sl]),
                "wqk0": np.concatenate(
                    [_pack_w(wq_g[:, 0:128]), _pack_w(wk_g[:, 0:128])], axis=1
                ),
                "wqk1": np.concatenate(
                    [_pack_w(wq_g[:, 128:256]), _pack_w(wk_g[:, 128:256])], axis=1
                ),
                "aux": aux,
            }
        )
    return in_maps


def kernel(x, Wq, bq, Wk, bk, Wv, bv):
    from concourse.bass_utils import run_bass_kernel_spmd

    nc = _get_runner()
    in_maps = _make_in_maps(x, Wq, bq, Wk, bk, Wv, bv)
    res = run_bass_kernel_spmd(nc, in_maps, list(range(N_CORES)))
    out = np.empty((B, S, D), dtype=np.float32)
    for c in range(N_CORES):
        b = c // 4
        g = c % 4
        out[b, :, DCORE * g : DCORE * (g + 1)] = res.results[c]["outT"].T
    return out
